# revision 1
# baseline (speedup 1.0000x reference)
"""Trainium2 Bass kernel for AdaptiveGraphConvolution (gnn_message_passing).

  pre_sup = x @ W                      [N, 64]
  s       = pre_sup[row] @ fw1 + pre_sup[col] @ fw2 + f_b     (per edge)
  deg     = bincount(row) + 1
  vals    = (deg[row] * deg[col]) ** (-s)
  out     = relu(segment_sum(vals[:, None] * pre_sup[col], row, N))

Strategy (8 NeuronCores, SPMD), core c owns destination nodes
[c*12500, (c+1)*12500):

  * Host does index-only preprocessing: edges bucketed by
    (dest-half, col-owner, 128-node dest window), padded to a cross-core
    uniform number of 128-edge tiles per bucket, sorted by col inside each
    bucket; indices localized (int16 owner-local col, window-local row).
  * Device phase A: pre_sup + per-node scalars via TensorE from x @ W_aug
    (W_aug carries W | W@fw2 | W@fw1); builds a 256B/row bf16 node table
    [vj(64) | l | b | 1 | b*l | 0...] and per-node row-side vector
    [a+fb | l | (a+fb)l | 1]; AllGather of the node table.
  * Phase B per (half, owner): one big SWDGE dma_gather (int16, owner-local)
    fetches per-edge source rows; per 128-edge tile the full 128x128
    (slot, node) weight matrix t = u(node)^T v(slot) comes from ONE rank-4
    TensorE matmul, ACT computes exp(-t), DVE masks by the one-hot of the
    edge's true dest row, and a second TensorE matmul (lhsT=masked weights,
    rhs=vj) accumulates the segment sum in a persistent PSUM block
    [128, 49 windows, 64]; ReLU + store at the end of the half.
"""

import sys

for _p in ("/opt/trn_rl_repo", "/opt/pypackages"):
    if _p not in sys.path:
        sys.path.append(_p)

import numpy as np
import ml_dtypes

import concourse.bass as bass
import concourse.bacc as bacc
import concourse.mybir as mybir
import concourse.tile as tile
from concourse.bass_utils import run_bass_kernel_spmd
from concourse.masks import make_identity

BF16 = ml_dtypes.bfloat16
P = 128
N_CORES = 8
ROWB = 128          # table row: 128 bf16 = 256B
CHUNK = 4           # tiles per exp/mask batch
N_SPLIT = 1         # gathers per (sweep, owner) block (SWDGE FIFO capacity)


# ----------------------------------------------------------------- host prep

def host_prep(row, col, n_nodes, n_cores):
    npc = n_nodes // n_cores
    npad = ((npc + P - 1) // P) * P
    n_win = npad // P
    n_sweep = 7 if n_win % 7 == 0 else (4 if n_win % 4 == 0 else 2)
    assert n_win % n_sweep == 0
    hw = n_win // n_sweep                            # windows per sweep

    row = np.asarray(row).astype(np.int64)
    col = np.asarray(col).astype(np.int64)
    order = np.argsort(row, kind="stable")
    row_s = row[order].astype(np.int32)
    col_s = col[order].astype(np.int32)

    percore = []
    cnt = np.zeros((n_cores, n_cores, n_win), np.int64)   # [core, owner, window]
    for c in range(n_cores):
        base = c * npc
        lo = np.searchsorted(row_s, base, "left")
        hi = np.searchsorted(row_s, base + npc, "left")
        r = (row_s[lo:hi] - base).astype(np.int32)
        cc = col_s[lo:hi]
        o = (cc // npc).astype(np.int32)
        w = r // P
        np.add.at(cnt[c], (o, w), 1)
        percore.append((r, cc, o, w))
    # uniform tiles per (owner, window) bucket across cores
    B = np.maximum((cnt.max(axis=0) + P - 1) // P, 1)     # [owner, window]
    n_tiles = int(B.sum())
    n_slots = n_tiles * P
    # slot offset of bucket (o, w): layout [half][owner][window][tiles]
    tile_base = np.zeros((n_cores, n_win), np.int64)
    t0 = 0
    order_buckets = []
    for hh in range(n_sweep):
        for oo in range(n_cores):
            for wi in range(hh * hw, (hh + 1) * hw):
                tile_base[oo, wi] = t0
                order_buckets.append((oo, wi))
                t0 += int(B[oo, wi])
    assert t0 == n_tiles

    shards = []
    for c in range(n_cores):
        r, cc, o, w = percore[c]
        idx16 = np.zeros(n_slots, np.int16)
        rloc = np.full(n_slots, -1.0, np.float32)
        bo = np.lexsort((cc, w, o))                  # sort by (owner, window, col)
        r, cc, o, w = r[bo], cc[bo], o[bo], w[bo]
        # start offset of each bucket's edges in the sorted list
        key = o.astype(np.int64) * n_win + w
        starts = np.searchsorted(key, np.arange(n_cores * n_win))
        ends = np.searchsorted(key, np.arange(n_cores * n_win), "right")
        for oo in range(n_cores):
            for wi in range(n_win):
                a, b = int(starts[oo * n_win + wi]), int(ends[oo * n_win + wi])
                if a == b:
                    continue
                s0 = int(tile_base[oo, wi]) * P
                k = b - a
                idx16[s0:s0 + k] = (cc[a:b] % npc).astype(np.int16)
                rloc[s0:s0 + k] = (r[a:b] - wi * P).astype(np.float32)
        deg = (np.bincount(r, minlength=npad) + 1).astype(np.float32)
        shards.append(dict(
            idx16=np.tile(np.ascontiguousarray(
                idx16.reshape(n_slots // 16, 16).T), (8, 1)),   # [128, S/16]
            rloc=np.ascontiguousarray(rloc.reshape(n_tiles, P).T),  # [P, n_tiles]
            deg=deg.reshape(npad, 1),
        ))
    L = dict(npc=npc, npad=npad, n_win=n_win, hw=hw, n_sweep=n_sweep, B=B,
             tile_base=tile_base, n_tiles=n_tiles)
    return shards, L


# ------------------------------------------------------------- device program

def build_program(L, in_dim, out_dim, n_cores):
    npad, n_win, hw = L["npad"], L["n_win"], L["hw"]
    n_sweep = L["n_sweep"]
    B, tile_base, n_tiles = L["B"], L["tile_base"], L["n_tiles"]
    n_k = in_dim // P
    f32, bf16, i16 = mybir.dt.float32, mybir.dt.bfloat16, mybir.dt.int16

    nc = bacc.Bacc("TRN2", target_bir_lowering=False, debug=False,
                   num_devices=n_cores)

    xin = nc.declare_dram_parameter("xin", [npad, in_dim], f32, isOutput=False)
    degp = nc.declare_dram_parameter("deg", [npad, 1], f32, isOutput=False)
    wmat = nc.declare_dram_parameter("wmat", [in_dim, out_dim], f32, isOutput=False)
    fw12 = nc.declare_dram_parameter("fw12", [out_dim, 2], f32, isOutput=False)
    fbrep = nc.declare_dram_parameter("fbrep", [P, 1], f32, isOutput=False)
    idxp = nc.declare_dram_parameter("idx16", [P, n_tiles * P // 16], i16,
                                     isOutput=False)
    rlocp = nc.declare_dram_parameter("rloc", [P, n_tiles], f32, isOutput=False)
    outp = nc.declare_dram_parameter("out", [npad, out_dim], f32, isOutput=True)

    with tile.TileContext(nc) as tc:
        with (
            tc.tile_pool(name="dram", bufs=1, space="DRAM") as dpool,
            tc.tile_pool(name="const", bufs=1) as cpool,
        ):
            t_loc = dpool.tile([npad, ROWB], bf16)
            t2_loc = dpool.tile([npad, 4], bf16)
            t_glob = dpool.tile([n_cores * npad, ROWB], bf16)

            identity = cpool.tile([P, P], bf16)
            make_identity(nc, identity[:])
            iota_i = cpool.tile([P, P], mybir.dt.int32)
            nc.gpsimd.iota(iota_i[:], pattern=[[1, P]], channel_multiplier=0)
            iota_bf = cpool.tile([P, P], bf16)
            nc.vector.tensor_copy(iota_bf[:], iota_i[:])
            fb_sb = cpool.tile([P, 1], f32)
            nc.sync.dma_start(out=fb_sb[:], in_=fbrep[:, :])
            fw_sb = cpool.tile([out_dim, 2], f32)
            nc.sync.dma_start(out=fw_sb[:], in_=fw12[:, :])
            fw_bf = cpool.tile([out_dim, 2], bf16)
            nc.vector.tensor_copy(fw_bf[:], fw_sb[:])

            # W_aug = [W | W@fw2 | W@fw1]  bf16 [P, n_k, 66]
            w_aug = cpool.tile([P, n_k, 66], bf16)
            with (
                tc.tile_pool(name="wtmp", bufs=2) as wpool,
                tc.tile_pool(name="wps", bufs=2, space="PSUM") as wps,
            ):
                wf = wpool.tile([P, n_k, out_dim], f32)
                nc.sync.dma_start(
                    out=wf[:], in_=wmat[:, :].rearrange("(s p) f -> p s f", p=P))
                nc.vector.tensor_copy(w_aug[:, :, 0:out_dim], wf[:])
                for s in range(n_k):
                    pT = wps.tile([out_dim, P], bf16, space="PSUM", tag="pT")
                    nc.tensor.transpose(out=pT[:], in_=w_aug[:, s, 0:out_dim],
                                        identity=identity[:])
                    wT = wpool.tile([out_dim, P], bf16, tag="wT")
                    nc.vector.tensor_copy(wT[:], pT[:])
                    pab = wps.tile([P, 2], f32, space="PSUM", tag="pab")
                    nc.tensor.matmul(out=pab[:], lhsT=wT[:], rhs=fw_bf[:],
                                     start=True, stop=True)
                    nc.vector.tensor_copy(w_aug[:, s, 64:65], pab[:, 1:2])  # b
                    nc.vector.tensor_copy(w_aug[:, s, 65:66], pab[:, 0:1])  # a

            # phase A: T row [vj(64)|l|b|1|bl|0...]; T2 row [a'|l|a'l|1]
            with (
                tc.tile_pool(name="xa", bufs=3) as xa,
                tc.tile_pool(name="psa", bufs=2, space="PSUM") as psa,
            ):
                for i in range(n_win):
                    xf = xa.tile([P, in_dim], f32, tag="xf")
                    nc.sync.dma_start(out=xf[:], in_=xin[i * P:(i + 1) * P, :])
                    xb = xa.tile([P, in_dim], bf16, tag="xb")
                    nc.vector.tensor_copy(xb[:], xf[:])
                    pt = psa.tile([P, 66], f32, space="PSUM", tag="pt")
                    for s in range(n_k):
                        pxt = psa.tile([P, P], bf16, space="PSUM", tag="pxt")
                        nc.tensor.transpose(out=pxt[:], in_=xb[:, s * P:(s + 1) * P],
                                            identity=identity[:])
                        xT = xa.tile([P, P], bf16, tag="xT")
                        nc.scalar.copy(xT[:], pxt[:])
                        nc.tensor.matmul(out=pt[:], lhsT=xT[:], rhs=w_aug[:, s, :],
                                         start=(s == 0), stop=(s == n_k - 1))
                    dg = xa.tile([P, 1], f32, tag="dg")
                    nc.sync.dma_start(out=dg[:], in_=degp[i * P:(i + 1) * P, :])
                    ldg = xa.tile([P, 1], f32, tag="ldg")
                    nc.scalar.activation(ldg[:], dg[:],
                                         mybir.ActivationFunctionType.Ln)
                    bl = xa.tile([P, 1], f32, tag="bl")
                    nc.vector.tensor_mul(bl[:], pt[:, 64:65], ldg[:])
                    tt = xa.tile([P, ROWB], bf16, tag="tt")
                    nc.vector.memset(tt[:, 68:ROWB], 0.0)
                    nc.vector.tensor_copy(tt[:, 0:64], pt[:, 0:64])
                    nc.vector.tensor_copy(tt[:, 64:65], ldg[:])
                    nc.vector.tensor_copy(tt[:, 65:66], pt[:, 64:65])
                    nc.vector.memset(tt[:, 66:67], 1.0)
                    nc.vector.tensor_copy(tt[:, 67:68], bl[:])
                    ap_ = xa.tile([P, 1], f32, tag="ap_")
                    nc.vector.tensor_add(ap_[:], pt[:, 65:66], fb_sb[:])
                    al = xa.tile([P, 1], f32, tag="al")
                    nc.vector.tensor_mul(al[:], ap_[:], ldg[:])
                    t2t = xa.tile([P, 4], bf16, tag="t2t")
                    nc.vector.tensor_copy(t2t[:, 0:1], ap_[:])
                    nc.vector.tensor_copy(t2t[:, 1:2], ldg[:])
                    nc.vector.tensor_copy(t2t[:, 2:3], al[:])
                    nc.vector.memset(t2t[:, 3:4], 1.0)
                    nc.sync.dma_start(out=t_loc[i * P:(i + 1) * P, :], in_=tt[:])
                    nc.sync.dma_start(out=t2_loc[i * P:(i + 1) * P, :], in_=t2t[:])

            nc.gpsimd.collective_compute(
                "AllGather", mybir.AluOpType.bypass,
                replica_groups=[list(range(n_cores))],
                ins=[t_loc.opt()], outs=[t_glob.opt()],
            )

            # ---------------- phase B
            dma_sem = nc.alloc_semaphore("dg_dma")
            prep_sem = nc.alloc_semaphore("dg_prep")
            n_gather = 0

            with (
                tc.tile_pool(name="ub", bufs=1) as ub,
                tc.tile_pool(name="gb", bufs=3) as gb,
                tc.tile_pool(name="wb", bufs=3) as wbp,
                tc.tile_pool(name="psb", bufs=1, space="PSUM") as psb,
                tc.tile_pool(name="psc", bufs=2, space="PSUM") as psc,
                tc.tile_pool(name="psd", bufs=4, space="PSUM") as psd,
            ):
                for hh in range(n_sweep):
                    u_sb = ub.tile([4, hw * P], bf16, tag="u_sb")
                    for wi in range(hw):
                        gwin = hh * hw + wi
                        t2w = wbp.tile([P, 4], bf16, tag="t2w")
                        nc.sync.dma_start(
                            out=t2w[:], in_=t2_loc[gwin * P:(gwin + 1) * P, :])
                        put = psd.tile([CHUNK * 4, P], bf16, space="PSUM", tag="ptr")
                        nc.tensor.transpose(out=put[0:4, :], in_=t2w[:],
                                            identity=identity[:])
                        nc.scalar.copy(u_sb[:, wi * P:(wi + 1) * P], put[0:4, :])

                    po = psb.tile([P, hw, out_dim], f32, space="PSUM", tag="po")
                    nc.vector.memset(po[:], 0.0)

                    def wins_of(tid0_, nt_):
                        out = []
                        for oo_ in range(n_cores):
                            t_lo_ = int(tile_base[oo_, hh * hw])
                            for wi_ in range(hw):
                                for _b in range(int(B[oo_, hh * hw + wi_])):
                                    out.append(wi_)
                        # global (within sweep) tile index -> window
                        base_ = int(tile_base[0, hh * hw])
                        return out[tid0_ - base_:tid0_ - base_ + nt_]

                    def process_tiles(tg, nt, tid0, wins, rl):
                        for ch0 in range(0, nt, CHUNK):
                            m = min(CHUNK, nt - ch0)
                            pt_ = psc.tile([P, CHUNK, P], f32, space="PSUM",
                                           tag="pt_")
                            v_tiles = []
                            for q in range(m):
                                pvt = psd.tile([4, P], bf16,
                                               space="PSUM", tag="ptr")
                                nc.tensor.transpose(
                                    out=pvt[:],
                                    in_=tg[:, ch0 + q, 64:68],
                                    identity=identity[:])
                                v_q = wbp.tile([4, P], bf16, tag="v_sb")
                                if (ch0 + q) % 2 == 1:
                                    nc.scalar.copy(v_q[:], pvt[:])
                                else:
                                    nc.vector.tensor_copy(v_q[:], pvt[:])
                                v_tiles.append(v_q)
                            for q in range(m):
                                wq = wins[ch0 + q]
                                nc.tensor.matmul(
                                    out=pt_[:, q, :],
                                    lhsT=v_tiles[q][:],
                                    rhs=u_sb[:, wq * P:(wq + 1) * P],
                                    start=True, stop=True)
                            ex = wbp.tile([P, CHUNK, P], bf16, tag="ex")
                            nc.scalar.activation(
                                ex[:, 0:m, :], pt_[:, 0:m, :],
                                mybir.ActivationFunctionType.Exp, scale=-1.0)
                            # batched one-hot: (rloc bcast) == iota  [P, m, P]
                            mk = wbp.tile([P, CHUNK, P], bf16, tag="mk")
                            nc.vector.tensor_tensor(
                                out=mk[:, 0:m, :],
                                in0=rl[:, ch0:ch0 + m].to_broadcast([P, m, P]),
                                in1=iota_bf[:, None, :].to_broadcast([P, m, P]),
                                op=mybir.AluOpType.is_equal)
                            msk = wbp.tile([P, CHUNK, P], bf16, tag="msk")
                            nc.vector.tensor_mul(
                                msk[:, 0:m, :], mk[:, 0:m, :], ex[:, 0:m, :])
                            for q in range(m):
                                wq = wins[ch0 + q]
                                nc.tensor.matmul(
                                    out=po[:, wq, :],
                                    lhsT=msk[:, q, :],
                                    rhs=tg[:, ch0 + q, 0:64],
                                    start=False, stop=False,
                                    skip_group_check=True)

                    process_queue = []
                    pending_tg = None
                    blocks = []
                    for oo in range(n_cores):
                        t_lo = int(tile_base[oo, hh * hw])
                        nt_all = int(B[oo, hh * hw:(hh + 1) * hw].sum())
                        splits = np.array_split(np.arange(nt_all), N_SPLIT)
                        for sp in splits:
                            if len(sp):
                                blocks.append((oo, t_lo + int(sp[0]), len(sp)))
                    for oo, tid0, nt in blocks:
                        if True:
                            n_idx = nt * P
                            tg = gb.tile([P, nt, ROWB], bf16, tag="tg")
                            ixs = gb.tile([P, n_idx // 16], i16, tag="ixs")
                            c0 = tid0 * P // 16
                            nc.sync.dma_start(
                                out=ixs[:], in_=idxp[:, c0:c0 + n_idx // 16])
                            with tc.tile_critical(no_gpsimd_drain=True):
                                nc.gpsimd.dma_gather(
                                    out_ap=tg[:],
                                    in_ap=t_glob[oo * npad:(oo + 1) * npad, :],
                                    idxs_ap=ixs[:],
                                    num_idxs=n_idx, num_idxs_reg=n_idx,
                                    elem_size=ROWB, single_packet=False,
                                    prepare_only=True, sem=dma_sem,
                                ).then_inc(prep_sem, 1)
                                n_gather += 1
                                nc.gpsimd.wait_ge(prep_sem, n_gather)
                                nc.gpsimd.trigger_dma(count=1)
                                if pending_tg is not None:
                                    nc.vector.wait_ge(dma_sem, 16 * (n_gather - 1))
                                    nc.vector.tensor_copy(
                                        pending_tg[:, :, 63:64],
                                        pending_tg[:, :, 63:64])
                                    nc.vector.tensor_copy(
                                        pending_tg[:, :, 64:68],
                                        pending_tg[:, :, 64:68])
                            process_queue.append(
                                (tg, nt, tid0, wins_of(tid0, nt)))
                            pending_tg = tg
                            if len(process_queue) < 2:
                                continue
                            tg, nt, tid0, wins = process_queue.pop(0)

                            rl = gb.tile([P, nt], f32, tag="rl")
                            nc.sync.dma_start(
                                out=rl[:], in_=rlocp[:, tid0:tid0 + nt])
                            process_tiles(tg, nt, tid0, wins, rl)

                    # drain remaining gathers
                    for tg, nt, tid0, wins in process_queue:
                        with tc.tile_critical(no_gpsimd_drain=True):
                            nc.vector.wait_ge(dma_sem, 16 * n_gather)
                            nc.vector.tensor_copy(
                                tg[:, :, 63:64], tg[:, :, 63:64])
                            nc.vector.tensor_copy(
                                tg[:, :, 64:68], tg[:, :, 64:68])
                        rl = gb.tile([P, nt], f32, tag="rl")
                        nc.sync.dma_start(
                            out=rl[:], in_=rlocp[:, tid0:tid0 + nt])
                        process_tiles(tg, nt, tid0, wins, rl)
                    process_queue = []
                    pending_tg = None

                    for wi in range(hw):
                        gwin = hh * hw + wi
                        ob = wbp.tile([P, out_dim], f32, tag="ob")
                        nc.scalar.activation(ob[:], po[:, wi, :],
                                             mybir.ActivationFunctionType.Relu)
                        nc.sync.dma_start(
                            out=outp[gwin * P:(gwin + 1) * P, :], in_=ob[:])

    nc.compile()
    return nc


# ------------------------------------------------------------------ assemble

def make_in_maps(x, W_, f_w, f_b, shards, L, n_cores):
    npc, npad, in_dim = L["npc"], L["npad"], x.shape[1]
    fw12 = np.stack([f_w[:64, 0], f_w[64:, 0]], axis=1).astype(np.float32)
    fbrep = np.full((P, 1), np.float32(f_b[0]), np.float32)
    in_maps = []
    for c in range(n_cores):
        xsh = np.zeros((npad, in_dim), np.float32)
        xsh[:npc] = x[c * npc:(c + 1) * npc]
        in_maps.append({
            "xin": xsh,
            "deg": shards[c]["deg"],
            "wmat": np.ascontiguousarray(W_, np.float32),
            "fw12": fw12,
            "fbrep": fbrep,
            "idx16": shards[c]["idx16"],
            "rloc": shards[c]["rloc"],
        })
    return in_maps


def kernel(x, W, f_w, f_b, row, col, _profile=None):
    x = np.asarray(x, np.float32)
    W = np.asarray(W, np.float32)
    f_w = np.asarray(f_w, np.float32)
    f_b = np.asarray(f_b, np.float32)
    n = x.shape[0]

    shards, L = host_prep(row, col, n, N_CORES)
    nc = build_program(L, x.shape[1], 64, N_CORES)
    in_maps = make_in_maps(x, W, f_w, f_b, shards, L, N_CORES)
    res = run_bass_kernel_spmd(
        nc, in_maps, core_ids=list(range(N_CORES)), trace=_profile is not None)
    if _profile is not None and isinstance(_profile, dict):
        _profile["exec_time_ns"] = res.exec_time_ns
        _profile["mean_exec_time_ns"] = res.mean_exec_time_ns

    npc = L["npc"]
    out = np.empty((n, 64), np.float32)
    for c in range(N_CORES):
        out[c * npc:(c + 1) * npc] = res.results[c]["out"][:npc]
    return out



# revision 2
# speedup vs baseline: 1.0762x; 1.0762x over previous
"""Trainium2 Bass kernel for AdaptiveGraphConvolution (gnn_message_passing).

  pre_sup = x @ W                      [N, 64]
  s       = pre_sup[row] @ fw1 + pre_sup[col] @ fw2 + f_b     (per edge)
  deg     = bincount(row) + 1
  vals    = (deg[row] * deg[col]) ** (-s)
  out     = relu(segment_sum(vals[:, None] * pre_sup[col], row, N))

Strategy (8 NeuronCores, SPMD), core c owns destination nodes
[c*12500, (c+1)*12500):

  * Host does index-only preprocessing: edges bucketed by
    (dest-half, col-owner, 128-node dest window), padded to a cross-core
    uniform number of 128-edge tiles per bucket, sorted by col inside each
    bucket; indices localized (int16 owner-local col, window-local row).
  * Device phase A: pre_sup + per-node scalars via TensorE from x @ W_aug
    (W_aug carries W | W@fw2 | W@fw1); builds a 256B/row bf16 node table
    [vj(64) | l | b | 1 | b*l | 0...] and per-node row-side vector
    [a+fb | l | (a+fb)l | 1]; AllGather of the node table.
  * Phase B per (half, owner): one big SWDGE dma_gather (int16, owner-local)
    fetches per-edge source rows; per 128-edge tile the full 128x128
    (slot, node) weight matrix t = u(node)^T v(slot) comes from ONE rank-4
    TensorE matmul, ACT computes exp(-t), DVE masks by the one-hot of the
    edge's true dest row, and a second TensorE matmul (lhsT=masked weights,
    rhs=vj) accumulates the segment sum in a persistent PSUM block
    [128, 49 windows, 64]; ReLU + store at the end of the half.
"""

import sys

for _p in ("/opt/trn_rl_repo", "/opt/pypackages"):
    if _p not in sys.path:
        sys.path.append(_p)

import numpy as np
import ml_dtypes

import concourse.bass as bass
import concourse.bacc as bacc
import concourse.mybir as mybir
import concourse.tile as tile
from concourse.bass_utils import run_bass_kernel_spmd
from concourse.masks import make_identity

BF16 = ml_dtypes.bfloat16
P = 128
N_CORES = 8
ROWB = 128          # table row: 128 bf16 = 256B
CHUNK = 4           # tiles per exp/mask batch
N_SPLIT = 1         # gathers per (sweep, owner) block (SWDGE FIFO capacity)


# ----------------------------------------------------------------- host prep

def host_prep(row, col, n_nodes, n_cores):
    npc = n_nodes // n_cores
    npad = ((npc + P - 1) // P) * P
    n_win = npad // P
    n_sweep = 7 if n_win % 7 == 0 else (4 if n_win % 4 == 0 else 2)
    assert n_win % n_sweep == 0
    hw = n_win // n_sweep                            # windows per sweep

    row = np.asarray(row).astype(np.int64)
    col = np.asarray(col).astype(np.int64)
    order = np.argsort(row, kind="stable")
    row_s = row[order].astype(np.int32)
    col_s = col[order].astype(np.int32)

    percore = []
    cnt = np.zeros((n_cores, n_cores, n_win), np.int64)   # [core, owner, window]
    for c in range(n_cores):
        base = c * npc
        lo = np.searchsorted(row_s, base, "left")
        hi = np.searchsorted(row_s, base + npc, "left")
        r = (row_s[lo:hi] - base).astype(np.int32)
        cc = col_s[lo:hi]
        o = (cc // npc).astype(np.int32)
        w = r // P
        np.add.at(cnt[c], (o, w), 1)
        percore.append((r, cc, o, w))
    # uniform tiles per (owner, window) bucket across cores
    B = np.maximum((cnt.max(axis=0) + P - 1) // P, 1)     # [owner, window]
    n_tiles = int(B.sum())
    n_slots = n_tiles * P
    # slot offset of bucket (o, w): layout [half][owner][window][tiles]
    tile_base = np.zeros((n_cores, n_win), np.int64)
    t0 = 0
    order_buckets = []
    for hh in range(n_sweep):
        for oo in range(n_cores):
            for wi in range(hh * hw, (hh + 1) * hw):
                tile_base[oo, wi] = t0
                order_buckets.append((oo, wi))
                t0 += int(B[oo, wi])
    assert t0 == n_tiles

    shards = []
    for c in range(n_cores):
        r, cc, o, w = percore[c]
        idx16 = np.zeros(n_slots, np.int16)
        rloc = np.full(n_slots, -1.0, np.float32)
        bo = np.lexsort((cc, w, o))                  # sort by (owner, window, col)
        r, cc, o, w = r[bo], cc[bo], o[bo], w[bo]
        # start offset of each bucket's edges in the sorted list
        key = o.astype(np.int64) * n_win + w
        starts = np.searchsorted(key, np.arange(n_cores * n_win))
        ends = np.searchsorted(key, np.arange(n_cores * n_win), "right")
        for oo in range(n_cores):
            for wi in range(n_win):
                a, b = int(starts[oo * n_win + wi]), int(ends[oo * n_win + wi])
                if a == b:
                    continue
                s0 = int(tile_base[oo, wi]) * P
                k = b - a
                idx16[s0:s0 + k] = (cc[a:b] % npc).astype(np.int16)
                rloc[s0:s0 + k] = (r[a:b] - wi * P).astype(np.float32)
        deg = (np.bincount(r, minlength=npad) + 1).astype(np.float32)
        shards.append(dict(
            idx16=np.tile(np.ascontiguousarray(
                idx16.reshape(n_slots // 16, 16).T), (8, 1)),   # [128, S/16]
            rloc=np.ascontiguousarray(rloc.reshape(n_tiles, P).T),  # [P, n_tiles]
            deg=deg.reshape(npad, 1),
        ))
    L = dict(npc=npc, npad=npad, n_win=n_win, hw=hw, n_sweep=n_sweep, B=B,
             tile_base=tile_base, n_tiles=n_tiles)
    return shards, L


# ------------------------------------------------------------- device program

def build_program(L, in_dim, out_dim, n_cores):
    npad, n_win, hw = L["npad"], L["n_win"], L["hw"]
    n_sweep = L["n_sweep"]
    B, tile_base, n_tiles = L["B"], L["tile_base"], L["n_tiles"]
    n_k = in_dim // P
    f32, bf16, i16 = mybir.dt.float32, mybir.dt.bfloat16, mybir.dt.int16

    nc = bacc.Bacc("TRN2", target_bir_lowering=False, debug=False,
                   num_devices=n_cores)

    xin = nc.declare_dram_parameter("xin", [npad, in_dim], f32, isOutput=False)
    degp = nc.declare_dram_parameter("deg", [npad, 1], f32, isOutput=False)
    wmat = nc.declare_dram_parameter("wmat", [in_dim, out_dim], f32, isOutput=False)
    fw12 = nc.declare_dram_parameter("fw12", [out_dim, 2], f32, isOutput=False)
    fbrep = nc.declare_dram_parameter("fbrep", [P, 1], f32, isOutput=False)
    idxp = nc.declare_dram_parameter("idx16", [P, n_tiles * P // 16], i16,
                                     isOutput=False)
    rlocp = nc.declare_dram_parameter("rloc", [P, n_tiles], f32, isOutput=False)
    outp = nc.declare_dram_parameter("out", [npad, out_dim], f32, isOutput=True)

    with tile.TileContext(nc) as tc:
        with (
            tc.tile_pool(name="dram", bufs=1, space="DRAM") as dpool,
            tc.tile_pool(name="const", bufs=1) as cpool,
        ):
            t_loc = dpool.tile([npad, ROWB], bf16)
            t2_loc = dpool.tile([npad, 4], bf16)
            t_glob = dpool.tile([n_cores * npad, ROWB], bf16)

            identity = cpool.tile([P, P], bf16)
            make_identity(nc, identity[:])
            iota_i = cpool.tile([P, P], mybir.dt.int32)
            nc.gpsimd.iota(iota_i[:], pattern=[[1, P]], channel_multiplier=0)
            iota_bf = cpool.tile([P, P], bf16)
            nc.vector.tensor_copy(iota_bf[:], iota_i[:])
            fb_sb = cpool.tile([P, 1], f32)
            nc.sync.dma_start(out=fb_sb[:], in_=fbrep[:, :])
            fw_sb = cpool.tile([out_dim, 2], f32)
            nc.sync.dma_start(out=fw_sb[:], in_=fw12[:, :])
            fw_bf = cpool.tile([out_dim, 2], bf16)
            nc.vector.tensor_copy(fw_bf[:], fw_sb[:])

            # W_aug = [W | W@fw2 | W@fw1]  bf16 [P, n_k, 66]
            w_aug = cpool.tile([P, n_k, 66], bf16)
            with (
                tc.tile_pool(name="wtmp", bufs=2) as wpool,
                tc.tile_pool(name="wps", bufs=2, space="PSUM") as wps,
            ):
                wf = wpool.tile([P, n_k, out_dim], f32)
                nc.sync.dma_start(
                    out=wf[:], in_=wmat[:, :].rearrange("(s p) f -> p s f", p=P))
                nc.vector.tensor_copy(w_aug[:, :, 0:out_dim], wf[:])
                for s in range(n_k):
                    pT = wps.tile([out_dim, P], bf16, space="PSUM", tag="pT")
                    nc.tensor.transpose(out=pT[:], in_=w_aug[:, s, 0:out_dim],
                                        identity=identity[:])
                    wT = wpool.tile([out_dim, P], bf16, tag="wT")
                    nc.vector.tensor_copy(wT[:], pT[:])
                    pab = wps.tile([P, 2], f32, space="PSUM", tag="pab")
                    nc.tensor.matmul(out=pab[:], lhsT=wT[:], rhs=fw_bf[:],
                                     start=True, stop=True)
                    nc.vector.tensor_copy(w_aug[:, s, 64:65], pab[:, 1:2])  # b
                    nc.vector.tensor_copy(w_aug[:, s, 65:66], pab[:, 0:1])  # a

            # phase A: T row [vj(64)|l|b|1|bl|0...]; T2 row [a'|l|a'l|1]
            with (
                tc.tile_pool(name="xa", bufs=3) as xa,
                tc.tile_pool(name="psa", bufs=2, space="PSUM") as psa,
            ):
                for i in range(n_win):
                    xf = xa.tile([P, in_dim], f32, tag="xf")
                    nc.sync.dma_start(out=xf[:], in_=xin[i * P:(i + 1) * P, :])
                    xb = xa.tile([P, in_dim], bf16, tag="xb")
                    nc.vector.tensor_copy(xb[:], xf[:])
                    pt = psa.tile([P, 66], f32, space="PSUM", tag="pt")
                    for s in range(n_k):
                        pxt = psa.tile([P, P], bf16, space="PSUM", tag="pxt")
                        nc.tensor.transpose(out=pxt[:], in_=xb[:, s * P:(s + 1) * P],
                                            identity=identity[:])
                        xT = xa.tile([P, P], bf16, tag="xT")
                        nc.scalar.copy(xT[:], pxt[:])
                        nc.tensor.matmul(out=pt[:], lhsT=xT[:], rhs=w_aug[:, s, :],
                                         start=(s == 0), stop=(s == n_k - 1))
                    dg = xa.tile([P, 1], f32, tag="dg")
                    nc.sync.dma_start(out=dg[:], in_=degp[i * P:(i + 1) * P, :])
                    ldg = xa.tile([P, 1], f32, tag="ldg")
                    nc.scalar.activation(ldg[:], dg[:],
                                         mybir.ActivationFunctionType.Ln)
                    bl = xa.tile([P, 1], f32, tag="bl")
                    nc.vector.tensor_mul(bl[:], pt[:, 64:65], ldg[:])
                    tt = xa.tile([P, ROWB], bf16, tag="tt")
                    nc.vector.memset(tt[:, 68:ROWB], 0.0)
                    nc.vector.tensor_copy(tt[:, 0:64], pt[:, 0:64])
                    nc.vector.tensor_copy(tt[:, 64:65], ldg[:])
                    nc.vector.tensor_copy(tt[:, 65:66], pt[:, 64:65])
                    nc.vector.memset(tt[:, 66:67], 1.0)
                    nc.vector.tensor_copy(tt[:, 67:68], bl[:])
                    ap_ = xa.tile([P, 1], f32, tag="ap_")
                    nc.vector.tensor_add(ap_[:], pt[:, 65:66], fb_sb[:])
                    al = xa.tile([P, 1], f32, tag="al")
                    nc.vector.tensor_mul(al[:], ap_[:], ldg[:])
                    t2t = xa.tile([P, 4], bf16, tag="t2t")
                    nc.vector.tensor_copy(t2t[:, 0:1], ap_[:])
                    nc.vector.tensor_copy(t2t[:, 1:2], ldg[:])
                    nc.vector.tensor_copy(t2t[:, 2:3], al[:])
                    nc.vector.memset(t2t[:, 3:4], 1.0)
                    nc.sync.dma_start(out=t_loc[i * P:(i + 1) * P, :], in_=tt[:])
                    nc.sync.dma_start(out=t2_loc[i * P:(i + 1) * P, :], in_=t2t[:])

            nc.gpsimd.collective_compute(
                "AllGather", mybir.AluOpType.bypass,
                replica_groups=[list(range(n_cores))],
                ins=[t_loc.opt()], outs=[t_glob.opt()],
            )

            # ---------------- phase B
            dma_sem = nc.alloc_semaphore("dg_dma")
            prep_sem = nc.alloc_semaphore("dg_prep")

            with (
                tc.tile_pool(name="ub", bufs=2) as ub,
                tc.tile_pool(name="tga", bufs=1) as tg_a,
                tc.tile_pool(name="tgb", bufs=1) as tg_b,
                tc.tile_pool(name="tgc", bufs=1) as tg_c,
                tc.tile_pool(name="tgd", bufs=1) as tg_d,
                tc.tile_pool(name="tge", bufs=1) as tg_e,
                tc.tile_pool(name="ixp", bufs=4) as ixp,
                tc.tile_pool(name="rlp", bufs=2) as rlp,
                tc.tile_pool(name="wb", bufs=3) as wbp,
                tc.tile_pool(name="psb", bufs=1, space="PSUM") as psb,
                tc.tile_pool(name="psc", bufs=2, space="PSUM") as psc,
                tc.tile_pool(name="psd", bufs=4, space="PSUM") as psd,
            ):
                tg_pools = [tg_a, tg_b, tg_c, tg_d, tg_e]

                # flat cross-sweep block list: (sweep, owner, tid0, nt)
                all_blocks = []
                for hh in range(n_sweep):
                    for oo in range(n_cores):
                        t_lo = int(tile_base[oo, hh * hw])
                        nt_all = int(B[oo, hh * hw:(hh + 1) * hw].sum())
                        if nt_all:
                            all_blocks.append((hh, oo, t_lo, nt_all))
                NB = len(all_blocks)

                def wins_of(hh_, tid0_, nt_):
                    out = []
                    for oo_ in range(n_cores):
                        for wi_ in range(hw):
                            for _b in range(int(B[oo_, hh_ * hw + wi_])):
                                out.append(wi_)
                    base_ = int(tile_base[0, hh_ * hw])
                    return out[tid0_ - base_:tid0_ - base_ + nt_]

                # prep side: descriptor-gen + trigger in a Pool-only critical.
                # The same critical publishes tg of block j-2 (whose DMA drain
                # finished during the previous gen) via gpsimd wait + dummy
                # copies, so no other engine ever appears in these criticals
                # and descriptor-gen pipelines with drain and compute.
                tg_tiles = [None] * NB

                def prep_block(j):
                    hh, oo, tid0, nt = all_blocks[j]
                    n_idx = nt * P
                    tg = tg_pools[j % 5].tile([P, nt, ROWB], bf16, tag="tg")
                    tg_tiles[j] = tg
                    ixs = ixp.tile([P, n_idx // 16], i16, tag="ixs")
                    c0 = tid0 * P // 16
                    nc.sync.dma_start(
                        out=ixs[:], in_=idxp[:, c0:c0 + n_idx // 16])
                    with tc.tile_critical(no_gpsimd_drain=True):
                        if j >= 2:
                            tgp = tg_tiles[j - 2]
                            nc.gpsimd.wait_ge(dma_sem, 16 * (j - 1))
                            nc.gpsimd.tensor_copy(tgp[:, :, 63:64],
                                                  tgp[:, :, 63:64])
                            nc.gpsimd.tensor_copy(tgp[:, :, 64:68],
                                                  tgp[:, :, 64:68])
                        nc.gpsimd.dma_gather(
                            out_ap=tg[:],
                            in_ap=t_glob[oo * npad:(oo + 1) * npad, :],
                            idxs_ap=ixs[:],
                            num_idxs=n_idx, num_idxs_reg=n_idx,
                            elem_size=ROWB, single_packet=False,
                            prepare_only=True, sem=dma_sem,
                        ).then_inc(prep_sem, 1)
                        nc.gpsimd.wait_ge(prep_sem, j + 1)
                        nc.gpsimd.trigger_dma(count=1)
                    return tg

                # tail publisher for the last two blocks (no prep to ride on)
                def publish_block(j):
                    tgp = tg_tiles[j]
                    with tc.tile_critical(no_gpsimd_drain=True):
                        nc.gpsimd.wait_ge(dma_sem, 16 * (j + 1))
                        nc.gpsimd.tensor_copy(tgp[:, :, 63:64],
                                              tgp[:, :, 63:64])
                        nc.gpsimd.tensor_copy(tgp[:, :, 64:68],
                                              tgp[:, :, 64:68])

                sweep_state = {}

                def sweep_setup(hh):
                    u_sb = ub.tile([4, hw * P], bf16, tag="u_sb")
                    for wi in range(hw):
                        gwin = hh * hw + wi
                        t2w = wbp.tile([P, 4], bf16, tag="t2w")
                        nc.sync.dma_start(
                            out=t2w[:], in_=t2_loc[gwin * P:(gwin + 1) * P, :])
                        put = psd.tile([CHUNK * 4, P], bf16, space="PSUM", tag="ptr")
                        nc.tensor.transpose(out=put[0:4, :], in_=t2w[:],
                                            identity=identity[:])
                        nc.scalar.copy(u_sb[:, wi * P:(wi + 1) * P], put[0:4, :])
                    po = psb.tile([P, hw, out_dim], f32, space="PSUM", tag="po")
                    nc.vector.memset(po[:], 0.0)
                    sweep_state[hh] = (u_sb, po)

                def sweep_store(hh):
                    _, po = sweep_state[hh]
                    for wi in range(hw):
                        gwin = hh * hw + wi
                        ob = wbp.tile([P, out_dim], f32, tag="ob")
                        nc.scalar.activation(ob[:], po[:, wi, :],
                                             mybir.ActivationFunctionType.Relu)
                        nc.sync.dma_start(
                            out=outp[gwin * P:(gwin + 1) * P, :], in_=ob[:])

                def process_tiles(hh, tg, nt, tid0, wins, rl):
                    u_sb, po = sweep_state[hh]
                    for ch0 in range(0, nt, CHUNK):
                        m = min(CHUNK, nt - ch0)
                        pt_ = psc.tile([P, CHUNK, P], f32, space="PSUM",
                                       tag="pt_")
                        v_tiles = []
                        for q in range(m):
                            pvt = psd.tile([4, P], bf16,
                                           space="PSUM", tag="ptr")
                            nc.tensor.transpose(
                                out=pvt[:],
                                in_=tg[:, ch0 + q, 64:68],
                                identity=identity[:])
                            v_q = wbp.tile([4, P], bf16, tag="v_sb")
                            if (ch0 + q) % 2 == 1:
                                nc.scalar.copy(v_q[:], pvt[:])
                            else:
                                nc.vector.tensor_copy(v_q[:], pvt[:])
                            v_tiles.append(v_q)
                        for q in range(m):
                            wq = wins[ch0 + q]
                            nc.tensor.matmul(
                                out=pt_[:, q, :],
                                lhsT=v_tiles[q][:],
                                rhs=u_sb[:, wq * P:(wq + 1) * P],
                                start=True, stop=True)
                        ex = wbp.tile([P, CHUNK, P], bf16, tag="ex")
                        nc.scalar.activation(
                            ex[:, 0:m, :], pt_[:, 0:m, :],
                            mybir.ActivationFunctionType.Exp, scale=-1.0)
                        # batched one-hot: (rloc bcast) == iota  [P, m, P]
                        mk = wbp.tile([P, CHUNK, P], bf16, tag="mk")
                        nc.vector.tensor_tensor(
                            out=mk[:, 0:m, :],
                            in0=rl[:, ch0:ch0 + m].to_broadcast([P, m, P]),
                            in1=iota_bf[:, None, :].to_broadcast([P, m, P]),
                            op=mybir.AluOpType.is_equal)
                        msk = wbp.tile([P, CHUNK, P], bf16, tag="msk")
                        nc.vector.tensor_mul(
                            msk[:, 0:m, :], mk[:, 0:m, :], ex[:, 0:m, :])
                        for q in range(m):
                            wq = wins[ch0 + q]
                            nc.tensor.matmul(
                                out=po[:, wq, :],
                                lhsT=msk[:, q, :],
                                rhs=tg[:, ch0 + q, 0:64],
                                start=False, stop=False,
                                skip_group_check=True)

                prep_block(0)
                prep_block(1)
                for j in range(NB):
                    if j + 2 < NB:
                        prep_block(j + 2)      # also publishes tg(j)
                    else:
                        publish_block(j)
                    hh, oo, tid0, nt = all_blocks[j]
                    if oo == 0:
                        sweep_setup(hh)
                    rl = rlp.tile([P, nt], f32, tag="rl")
                    nc.sync.dma_start(
                        out=rl[:], in_=rlocp[:, tid0:tid0 + nt])
                    process_tiles(hh, tg_tiles[j], nt, tid0,
                                  wins_of(hh, tid0, nt), rl)
                    tg_tiles[j] = None
                    if oo == n_cores - 1:
                        sweep_store(hh)

    nc.compile()
    return nc


# ------------------------------------------------------------------ assemble

def make_in_maps(x, W_, f_w, f_b, shards, L, n_cores):
    npc, npad, in_dim = L["npc"], L["npad"], x.shape[1]
    fw12 = np.stack([f_w[:64, 0], f_w[64:, 0]], axis=1).astype(np.float32)
    fbrep = np.full((P, 1), np.float32(f_b[0]), np.float32)
    in_maps = []
    for c in range(n_cores):
        xsh = np.zeros((npad, in_dim), np.float32)
        xsh[:npc] = x[c * npc:(c + 1) * npc]
        in_maps.append({
            "xin": xsh,
            "deg": shards[c]["deg"],
            "wmat": np.ascontiguousarray(W_, np.float32),
            "fw12": fw12,
            "fbrep": fbrep,
            "idx16": shards[c]["idx16"],
            "rloc": shards[c]["rloc"],
        })
    return in_maps


def kernel(x, W, f_w, f_b, row, col, _profile=None):
    x = np.asarray(x, np.float32)
    W = np.asarray(W, np.float32)
    f_w = np.asarray(f_w, np.float32)
    f_b = np.asarray(f_b, np.float32)
    n = x.shape[0]

    shards, L = host_prep(row, col, n, N_CORES)
    nc = build_program(L, x.shape[1], 64, N_CORES)
    in_maps = make_in_maps(x, W, f_w, f_b, shards, L, N_CORES)
    res = run_bass_kernel_spmd(
        nc, in_maps, core_ids=list(range(N_CORES)), trace=_profile is not None)
    if _profile is not None and isinstance(_profile, dict):
        _profile["exec_time_ns"] = res.exec_time_ns
        _profile["mean_exec_time_ns"] = res.mean_exec_time_ns

    npc = L["npc"]
    out = np.empty((n, 64), np.float32)
    for c in range(N_CORES):
        out[c * npc:(c + 1) * npc] = res.results[c]["out"][:npc]
    return out



# revision 3
# speedup vs baseline: 1.2412x; 1.1533x over previous
"""Trainium2 Bass kernel for AdaptiveGraphConvolution (gnn_message_passing).

  pre_sup = x @ W                      [N, 64]
  s       = pre_sup[row] @ fw1 + pre_sup[col] @ fw2 + f_b     (per edge)
  deg     = bincount(row) + 1
  vals    = (deg[row] * deg[col]) ** (-s)
  out     = relu(segment_sum(vals[:, None] * pre_sup[col], row, N))

Strategy (8 NeuronCores, SPMD), core c owns destination nodes
[c*12500, (c+1)*12500):

  * Host does index-only preprocessing: edges bucketed by
    (col-owner, 128-node dest window), padded to a cross-core uniform
    number of 128-edge tiles per bucket; indices localized (int16
    owner-local col). Per-slot mask features encode the window-local dest
    row r as exact-bf16 quadratic pairs (h=r>>3, l=r&7) so the one-hot
    mask is folded into the edge matmul (rank-10) and exp(-t') yields the
    masked edge weight directly (mismatched rows get penalty >= 32 in the
    exponent -> exp ~ 0; padding slots get +30000).
  * Device phase A: pre_sup + per-node scalars via TensorE from
    host-transposed x @ W_aug (W_aug carries W | W@fw2 | W@fw1), batched
    4 windows per iteration; builds a 256B/row bf16 node table
    [vj(64) | l | b | 1 | b*l | 0...] and per-node row-side vector
    [a+fb | l | (a+fb)l | 1]; AllGather of the node table.
  * Phase B per (sweep, owner): SWDGE dma_gather (int16, owner-local)
    fetches per-edge source rows; prep/trigger/publish all run in
    Pool-only critical sections so descriptor-gen overlaps the previous
    gather's drain and the tile compute; per 128-edge tile ONE rank-10
    TensorE matmul produces the masked exponent grid, ACT exp gives the
    masked weights, and a second TensorE matmul (lhsT=masked weights,
    rhs=vj) accumulates the segment sum in a persistent PSUM block
    [128, 14 windows, 64]; ReLU + store at the end of the sweep.
"""

import sys

for _p in ("/opt/trn_rl_repo", "/opt/pypackages"):
    if _p not in sys.path:
        sys.path.append(_p)

import numpy as np
import ml_dtypes

import concourse.bass as bass
import concourse.bacc as bacc
import concourse.mybir as mybir
import concourse.tile as tile
from concourse.bass_utils import run_bass_kernel_spmd
from concourse.masks import make_identity

BF16 = ml_dtypes.bfloat16
P = 128
N_CORES = 8
ROWB = 128          # table row: 128 bf16 = 256B
CHUNK = 4           # tiles per exp batch
WGRP = 4            # windows per phase-A iteration
MPEN = 32.0         # mask mismatch penalty scale (exact in bf16)
MBIG = 30000.0      # padding-slot penalty


# ----------------------------------------------------------------- host prep

def host_prep(row, col, n_nodes, n_cores):
    npc = n_nodes // n_cores
    npad = ((npc + P - 1) // P) * P
    n_win = npad // P
    n_sweep = 7 if n_win % 7 == 0 else (4 if n_win % 4 == 0 else 2)
    assert n_win % n_sweep == 0
    hw = n_win // n_sweep                            # windows per sweep

    row = np.asarray(row).astype(np.int64)
    col = np.asarray(col).astype(np.int64)
    order = np.argsort(row, kind="stable")
    row_s = row[order].astype(np.int32)
    col_s = col[order].astype(np.int32)

    percore = []
    cnt = np.zeros((n_cores, n_cores, n_win), np.int64)   # [core, owner, window]
    for c in range(n_cores):
        base = c * npc
        lo = np.searchsorted(row_s, base, "left")
        hi = np.searchsorted(row_s, base + npc, "left")
        r = (row_s[lo:hi] - base).astype(np.int32)
        cc = col_s[lo:hi]
        o = (cc // npc).astype(np.int32)
        w = r // P
        np.add.at(cnt[c], (o, w), 1)
        percore.append((r, cc, o, w))
    # uniform tiles per (owner, window) bucket across cores
    B = np.maximum((cnt.max(axis=0) + P - 1) // P, 1)     # [owner, window]
    n_tiles = int(B.sum())
    n_slots = n_tiles * P
    # slot offset of bucket (o, w): layout [sweep][owner][window][tiles]
    tile_base = np.zeros((n_cores, n_win), np.int64)
    t0 = 0
    for hh in range(n_sweep):
        for oo in range(n_cores):
            for wi in range(hh * hw, (hh + 1) * hw):
                tile_base[oo, wi] = t0
                t0 += int(B[oo, wi])
    assert t0 == n_tiles

    shards = []
    for c in range(n_cores):
        r, cc, o, w = percore[c]
        idx16 = np.zeros(n_slots, np.int16)
        rloc = np.full(n_slots, -1, np.int32)
        bo = np.lexsort((cc, w, o))                  # sort by (owner, window, col)
        r, cc, o, w = r[bo], cc[bo], o[bo], w[bo]
        key = o.astype(np.int64) * n_win + w
        starts = np.searchsorted(key, np.arange(n_cores * n_win))
        ends = np.searchsorted(key, np.arange(n_cores * n_win), "right")
        for oo in range(n_cores):
            for wi in range(n_win):
                a, b = int(starts[oo * n_win + wi]), int(ends[oo * n_win + wi])
                if a == b:
                    continue
                s0 = int(tile_base[oo, wi]) * P
                k = b - a
                idx16[s0:s0 + k] = (cc[a:b] % npc).astype(np.int16)
                rloc[s0:s0 + k] = (r[a:b] - wi * P)
        deg = (np.bincount(r, minlength=npad) + 1).astype(np.float32)
        # mask features [6, n_slots]: [Mh^2, -2Mh, M, Ml^2, -2Ml, M];
        # padding slots get [MBIG, 0, 0, 0, 0, 0]
        pad = rloc < 0
        rr = np.where(pad, 0, rloc)
        h = (rr >> 3).astype(np.float32)
        l = (rr & 7).astype(np.float32)
        vm = np.stack([
            MPEN * h * h, -2.0 * MPEN * h,
            np.full(n_slots, MPEN, np.float32),
            MPEN * l * l, -2.0 * MPEN * l,
            np.full(n_slots, MPEN, np.float32),
        ], axis=0)
        vm[0, pad] = MBIG
        vm[1:, pad] = 0.0
        shards.append(dict(
            idx16=np.tile(np.ascontiguousarray(
                idx16.reshape(n_slots // 16, 16).T), (8, 1)),   # [128, S/16]
            vmask=np.ascontiguousarray(vm.astype(BF16)),        # [6, n_slots]
            deg=deg.reshape(npad, 1),
        ))
    L = dict(npc=npc, npad=npad, n_win=n_win, hw=hw, n_sweep=n_sweep, B=B,
             tile_base=tile_base, n_tiles=n_tiles)
    return shards, L


# ------------------------------------------------------------- device program

def build_program(L, in_dim, out_dim, n_cores):
    npad, n_win, hw = L["npad"], L["n_win"], L["hw"]
    n_sweep = L["n_sweep"]
    B, tile_base, n_tiles = L["B"], L["tile_base"], L["n_tiles"]
    n_k = in_dim // P
    f32, bf16, i16 = mybir.dt.float32, mybir.dt.bfloat16, mybir.dt.int16

    nc = bacc.Bacc("TRN2", target_bir_lowering=False, debug=False,
                   num_devices=n_cores)

    xint = nc.declare_dram_parameter("xint", [in_dim, npad], f32, isOutput=False)
    degp = nc.declare_dram_parameter("deg", [npad, 1], f32, isOutput=False)
    wmat = nc.declare_dram_parameter("wmat", [in_dim, out_dim], f32, isOutput=False)
    fw12 = nc.declare_dram_parameter("fw12", [out_dim, 2], f32, isOutput=False)
    fbrep = nc.declare_dram_parameter("fbrep", [P, 1], f32, isOutput=False)
    idxp = nc.declare_dram_parameter("idx16", [P, n_tiles * P // 16], i16,
                                     isOutput=False)
    vmaskp = nc.declare_dram_parameter("vmask", [6, n_tiles * P], bf16,
                                       isOutput=False)
    u6p = nc.declare_dram_parameter("u6rep", [6, hw * P], bf16, isOutput=False)
    outp = nc.declare_dram_parameter("out", [npad, out_dim], f32, isOutput=True)
    idx_cols = n_tiles * P // 16

    with tile.TileContext(nc) as tc:
        with (
            tc.tile_pool(name="dram", bufs=1, space="DRAM") as dpool,
            tc.tile_pool(name="const", bufs=1) as cpool,
        ):
            t_loc = dpool.tile([npad, ROWB], bf16)
            t2_loc = dpool.tile([npad, 4], bf16)
            t_glob = dpool.tile([n_cores * npad, ROWB], bf16)

            identity = cpool.tile([P, P], bf16)
            make_identity(nc, identity[:])
            fb_sb = cpool.tile([P, 1], f32)
            nc.sync.dma_start(out=fb_sb[:], in_=fbrep[:, :])
            fw_sb = cpool.tile([out_dim, 2], f32)
            nc.sync.dma_start(out=fw_sb[:], in_=fw12[:, :])
            fw_bf = cpool.tile([out_dim, 2], bf16)
            nc.vector.tensor_copy(fw_bf[:], fw_sb[:])

            # W_aug = [W | W@fw2 | W@fw1]  bf16 [P, n_k, 66]
            w_aug = cpool.tile([P, n_k, 66], bf16)
            with (
                tc.tile_pool(name="wtmp", bufs=2) as wpool,
                tc.tile_pool(name="wps", bufs=2, space="PSUM") as wps,
            ):
                wf = wpool.tile([P, n_k, out_dim], f32)
                nc.sync.dma_start(
                    out=wf[:], in_=wmat[:, :].rearrange("(s p) f -> p s f", p=P))
                nc.vector.tensor_copy(w_aug[:, :, 0:out_dim], wf[:])
                for s in range(n_k):
                    pT = wps.tile([out_dim, P], bf16, space="PSUM", tag="pT")
                    nc.tensor.transpose(out=pT[:], in_=w_aug[:, s, 0:out_dim],
                                        identity=identity[:])
                    wT = wpool.tile([out_dim, P], bf16, tag="wT")
                    nc.vector.tensor_copy(wT[:], pT[:])
                    pab = wps.tile([P, 2], f32, space="PSUM", tag="pab")
                    nc.tensor.matmul(out=pab[:], lhsT=wT[:], rhs=fw_bf[:],
                                     start=True, stop=True)
                    nc.vector.tensor_copy(w_aug[:, s, 64:65], pab[:, 1:2])  # b
                    nc.vector.tensor_copy(w_aug[:, s, 65:66], pab[:, 0:1])  # a

            # phase A (batched WGRP windows): T row [vj(64)|l|b|1|bl|0...];
            # T2 row [a'|l|a'l|1]
            groups = []
            w0 = 0
            while w0 < n_win:
                wn = min(WGRP, n_win - w0)
                groups.append((w0, wn))
                w0 += wn
            with (
                tc.tile_pool(name="xa", bufs=3) as xa,
                tc.tile_pool(name="psa", bufs=2, space="PSUM") as psa,
            ):
                for (g0, gn) in groups:
                    xf = xa.tile([P, n_k, WGRP * P], f32, tag="xf")
                    for s in range(n_k):
                        nc.sync.dma_start(
                            out=xf[:, s, 0:gn * P],
                            in_=xint[s * P:(s + 1) * P, g0 * P:(g0 + gn) * P])
                    xb = xa.tile([P, n_k, WGRP * P], bf16, tag="xb")
                    nc.vector.tensor_copy(xb[:, :, 0:gn * P], xf[:, :, 0:gn * P])
                    pt = psa.tile([P, WGRP, 66], f32, space="PSUM", tag="pt")
                    for w in range(gn):
                        for s in range(n_k):
                            nc.tensor.matmul(
                                out=pt[:, w, :],
                                lhsT=xb[:, s, w * P:(w + 1) * P],
                                rhs=w_aug[:, s, :],
                                start=(s == 0), stop=(s == n_k - 1))
                    dg = xa.tile([P, WGRP, 1], f32, tag="dg")
                    nc.sync.dma_start(
                        out=dg[:, 0:gn, :],
                        in_=degp[g0 * P:(g0 + gn) * P, :].rearrange(
                            "(w p) o -> p w o", p=P))
                    ldg = xa.tile([P, WGRP, 1], f32, tag="ldg")
                    nc.scalar.activation(ldg[:, 0:gn, :], dg[:, 0:gn, :],
                                         mybir.ActivationFunctionType.Ln)
                    bl = xa.tile([P, WGRP, 1], f32, tag="bl")
                    nc.vector.tensor_mul(bl[:, 0:gn, :], pt[:, 0:gn, 64:65],
                                         ldg[:, 0:gn, :])
                    tt = xa.tile([P, WGRP, ROWB], bf16, tag="tt")
                    nc.vector.memset(tt[:, 0:gn, 68:ROWB], 0.0)
                    nc.vector.tensor_copy(tt[:, 0:gn, 0:64], pt[:, 0:gn, 0:64])
                    nc.vector.tensor_copy(tt[:, 0:gn, 64:65], ldg[:, 0:gn, :])
                    nc.vector.tensor_copy(tt[:, 0:gn, 65:66], pt[:, 0:gn, 64:65])
                    nc.vector.memset(tt[:, 0:gn, 66:67], 1.0)
                    nc.vector.tensor_copy(tt[:, 0:gn, 67:68], bl[:, 0:gn, :])
                    ap_ = xa.tile([P, WGRP, 1], f32, tag="ap_")
                    nc.vector.tensor_tensor(
                        out=ap_[:, 0:gn, :], in0=pt[:, 0:gn, 65:66],
                        in1=fb_sb[:, None, :].to_broadcast([P, gn, 1]),
                        op=mybir.AluOpType.add)
                    al = xa.tile([P, WGRP, 1], f32, tag="al")
                    nc.vector.tensor_mul(al[:, 0:gn, :], ap_[:, 0:gn, :],
                                         ldg[:, 0:gn, :])
                    t2t = xa.tile([P, WGRP, 4], bf16, tag="t2t")
                    nc.vector.tensor_copy(t2t[:, 0:gn, 0:1], ap_[:, 0:gn, :])
                    nc.vector.tensor_copy(t2t[:, 0:gn, 1:2], ldg[:, 0:gn, :])
                    nc.vector.tensor_copy(t2t[:, 0:gn, 2:3], al[:, 0:gn, :])
                    nc.vector.memset(t2t[:, 0:gn, 3:4], 1.0)
                    nc.sync.dma_start(
                        out=t_loc[g0 * P:(g0 + gn) * P, :].rearrange(
                            "(w p) f -> p w f", p=P),
                        in_=tt[:, 0:gn, :])
                    nc.sync.dma_start(
                        out=t2_loc[g0 * P:(g0 + gn) * P, :].rearrange(
                            "(w p) f -> p w f", p=P),
                        in_=t2t[:, 0:gn, :])

            nc.gpsimd.collective_compute(
                "AllGather", mybir.AluOpType.bypass,
                replica_groups=[list(range(n_cores))],
                ins=[t_loc.opt()], outs=[t_glob.opt()],
            )

            # ---------------- phase B
            dma_sem = nc.alloc_semaphore("dg_dma")
            prep_sem = nc.alloc_semaphore("dg_prep")

            with (
                tc.tile_pool(name="ub", bufs=2) as ub,
                tc.tile_pool(name="ixp", bufs=4) as ixp,
                tc.tile_pool(name="tga", bufs=1) as tg_a,
                tc.tile_pool(name="tgb", bufs=1) as tg_b,
                tc.tile_pool(name="tgc", bufs=1) as tg_c,
                tc.tile_pool(name="tgd", bufs=1) as tg_d,
                tc.tile_pool(name="tge", bufs=1) as tg_e,
                tc.tile_pool(name="vbp", bufs=4) as vbp,
                tc.tile_pool(name="wb", bufs=3) as wbp,
                tc.tile_pool(name="psb", bufs=1, space="PSUM") as psb,
                tc.tile_pool(name="psc", bufs=2, space="PSUM") as psc,
                tc.tile_pool(name="psd", bufs=4, space="PSUM") as psd,
            ):
                tg_pools = [tg_a, tg_b, tg_c, tg_d, tg_e]

                # flat cross-sweep block list: (sweep, owner, tid0, nt)
                all_blocks = []
                for hh in range(n_sweep):
                    for oo in range(n_cores):
                        t_lo = int(tile_base[oo, hh * hw])
                        nt_all = int(B[oo, hh * hw:(hh + 1) * hw].sum())
                        if nt_all:
                            all_blocks.append((hh, oo, t_lo, nt_all))
                NB = len(all_blocks)

                def wins_of(hh_, tid0_, nt_):
                    out = []
                    for oo_ in range(n_cores):
                        for wi_ in range(hw):
                            for _b in range(int(B[oo_, hh_ * hw + wi_])):
                                out.append(wi_)
                    base_ = int(tile_base[0, hh_ * hw])
                    return out[tid0_ - base_:tid0_ - base_ + nt_]

                # prep side: descriptor-gen + trigger in a Pool-only critical;
                # the publish (gpsimd wait for the gather DMA + dummy copies)
                # is its own tiny Pool critical issued BEFORE prep(j+2), so
                # compute(j) is released two criticals before any critical
                # whose entry barrier covers it: gen, drain and compute all
                # overlap.
                tg_tiles = [None] * NB

                def prep_block(j):
                    hh, oo, tid0, nt = all_blocks[j]
                    n_idx = nt * P
                    tg = tg_pools[j % 5].tile([P, nt, ROWB], bf16, tag="tg")
                    tg_tiles[j] = tg
                    ixs = ixp.tile([P, n_idx // 16], i16, tag="ixs")
                    c0 = tid0 * P // 16
                    nc.sync.dma_start(
                        out=ixs[:], in_=idxp[:, c0:c0 + n_idx // 16])
                    with tc.tile_critical(no_gpsimd_drain=True):
                        nc.gpsimd.dma_gather(
                            out_ap=tg[:],
                            in_ap=t_glob[oo * npad:(oo + 1) * npad, :],
                            idxs_ap=ixs[:],
                            num_idxs=n_idx, num_idxs_reg=n_idx,
                            elem_size=ROWB, single_packet=False,
                            prepare_only=True, sem=dma_sem,
                        ).then_inc(prep_sem, 1)
                        nc.gpsimd.wait_ge(prep_sem, j + 1)
                        nc.gpsimd.trigger_dma(count=1)
                    return tg

                def publish_block(j):
                    tgp = tg_tiles[j]
                    with tc.tile_critical(no_gpsimd_drain=True):
                        nc.gpsimd.wait_ge(dma_sem, 16 * (j + 1))
                        nc.gpsimd.tensor_copy(tgp[:, :, 63:64],
                                              tgp[:, :, 63:64])
                        nc.gpsimd.tensor_copy(tgp[:, :, 64:68],
                                              tgp[:, :, 64:68])

                sweep_state = {}

                def sweep_setup(hh):
                    u_sb = ub.tile([10, hw * P], bf16, tag="u_sb")
                    nc.sync.dma_start(out=u_sb[4:10, :], in_=u6p[:, :])
                    for wi in range(hw):
                        gwin = hh * hw + wi
                        t2w = wbp.tile([P, 4], bf16, tag="t2w")
                        nc.sync.dma_start(
                            out=t2w[:], in_=t2_loc[gwin * P:(gwin + 1) * P, :])
                        put = psd.tile([CHUNK * 4, P], bf16, space="PSUM", tag="ptr")
                        nc.tensor.transpose(out=put[0:4, :], in_=t2w[:],
                                            identity=identity[:])
                        nc.scalar.copy(u_sb[0:4, wi * P:(wi + 1) * P], put[0:4, :])
                    po = psb.tile([P, hw, out_dim], f32, space="PSUM", tag="po")
                    nc.vector.memset(po[:], 0.0)
                    sweep_state[hh] = (u_sb, po)

                def sweep_store(hh):
                    _, po = sweep_state[hh]
                    for wi in range(hw):
                        gwin = hh * hw + wi
                        ob = wbp.tile([P, out_dim], f32, tag="ob")
                        nc.scalar.activation(ob[:], po[:, wi, :],
                                             mybir.ActivationFunctionType.Relu)
                        nc.scalar.dma_start(
                            out=outp[gwin * P:(gwin + 1) * P, :], in_=ob[:])

                def process_tiles(hh, tg, nt, tid0, wins, v10):
                    u_sb, po = sweep_state[hh]
                    for ch0 in range(0, nt, CHUNK):
                        m = min(CHUNK, nt - ch0)
                        pt_ = psc.tile([P, CHUNK, P], f32, space="PSUM",
                                       tag="pt_")
                        for q in range(m):
                            pvt = psd.tile([4, P], bf16,
                                           space="PSUM", tag="ptr")
                            nc.tensor.transpose(
                                out=pvt[:],
                                in_=tg[:, ch0 + q, 64:68],
                                identity=identity[:])
                            dst = v10[0:4, (ch0 + q) * P:(ch0 + q + 1) * P]
                            nc.scalar.copy(dst, pvt[:])
                        for q in range(m):
                            wq = wins[ch0 + q]
                            nc.tensor.matmul(
                                out=pt_[:, q, :],
                                lhsT=v10[:, (ch0 + q) * P:(ch0 + q + 1) * P],
                                rhs=u_sb[:, wq * P:(wq + 1) * P],
                                start=True, stop=True)
                        # exp(-t') IS the masked weight grid
                        msk = wbp.tile([P, CHUNK, P], bf16, tag="msk")
                        nc.scalar.activation(
                            msk[:, 0:m, :], pt_[:, 0:m, :],
                            mybir.ActivationFunctionType.Exp, scale=-1.0)
                        for q in range(m):
                            wq = wins[ch0 + q]
                            nc.tensor.matmul(
                                out=po[:, wq, :],
                                lhsT=msk[:, q, :],
                                rhs=tg[:, ch0 + q, 0:64],
                                start=False, stop=False,
                                skip_group_check=True)

                prep_block(0)
                prep_block(1)
                for j in range(NB):
                    hh, oo, tid0, nt = all_blocks[j]
                    if oo == 0:
                        sweep_setup(hh)
                    v10 = vbp.tile([10, nt * P], bf16, tag="v10")
                    nc.sync.dma_start(
                        out=v10[4:10, :],
                        in_=vmaskp[:, tid0 * P:(tid0 + nt) * P])
                    publish_block(j)
                    if j + 2 < NB:
                        prep_block(j + 2)
                    process_tiles(hh, tg_tiles[j], nt, tid0,
                                  wins_of(hh, tid0, nt), v10)
                    tg_tiles[j] = None
                    if oo == n_cores - 1:
                        sweep_store(hh)

    nc.compile()
    return nc


# ------------------------------------------------------------------ assemble

def make_in_maps(x, W_, f_w, f_b, shards, L, n_cores):
    npc, npad, in_dim = L["npc"], L["npad"], x.shape[1]
    hw = L["hw"]
    fw12 = np.stack([f_w[:64, 0], f_w[64:, 0]], axis=1).astype(np.float32)
    fbrep = np.full((P, 1), np.float32(f_b[0]), np.float32)
    # u-side mask features per window row r: [1, h, h^2, 1, l, l^2]
    r = np.arange(P, dtype=np.float32)
    h = np.floor(r / 8.0)
    l = r - 8.0 * h
    u6 = np.stack([np.ones(P, np.float32), h, h * h,
                   np.ones(P, np.float32), l, l * l], axis=0)
    u6rep = np.ascontiguousarray(np.tile(u6, (1, hw)).astype(BF16))
    in_maps = []
    for c in range(n_cores):
        xsh = np.zeros((npad, in_dim), np.float32)
        xsh[:npc] = x[c * npc:(c + 1) * npc]
        xint = np.ascontiguousarray(xsh.T)
        in_maps.append({
            "xint": xint,
            "deg": shards[c]["deg"],
            "wmat": np.ascontiguousarray(W_, np.float32),
            "fw12": fw12,
            "fbrep": fbrep,
            "idx16": shards[c]["idx16"],
            "vmask": shards[c]["vmask"],
            "u6rep": u6rep,
        })
    return in_maps


def kernel(x, W, f_w, f_b, row, col, _profile=None):
    x = np.asarray(x, np.float32)
    W = np.asarray(W, np.float32)
    f_w = np.asarray(f_w, np.float32)
    f_b = np.asarray(f_b, np.float32)
    n = x.shape[0]

    shards, L = host_prep(row, col, n, N_CORES)
    nc = build_program(L, x.shape[1], 64, N_CORES)
    in_maps = make_in_maps(x, W, f_w, f_b, shards, L, N_CORES)
    res = run_bass_kernel_spmd(
        nc, in_maps, core_ids=list(range(N_CORES)), trace=_profile is not None)
    if _profile is not None and isinstance(_profile, dict):
        _profile["exec_time_ns"] = res.exec_time_ns
        _profile["mean_exec_time_ns"] = res.mean_exec_time_ns

    npc = L["npc"]
    out = np.empty((n, 64), np.float32)
    for c in range(N_CORES):
        out[c * npc:(c + 1) * npc] = res.results[c]["out"][:npc]
    return out


# revision 5
# speedup vs baseline: 1.3488x; 1.0867x over previous
"""Trainium2 Bass kernel for AdaptiveGraphConvolution (gnn_message_passing).

  pre_sup = x @ W                      [N, 64]
  s       = pre_sup[row] @ fw1 + pre_sup[col] @ fw2 + f_b     (per edge)
  deg     = bincount(row) + 1
  vals    = (deg[row] * deg[col]) ** (-s)
  out     = relu(segment_sum(vals[:, None] * pre_sup[col], row, N))

Strategy (8 NeuronCores, SPMD), core c owns destination nodes
[c*12500, (c+1)*12500):

  * Host does index-only preprocessing: edges bucketed by
    (col-owner, 128-node dest window), padded to a cross-core uniform
    number of 128-edge tiles per bucket; indices localized (int16
    owner-local col). Per-slot mask features encode the window-local dest
    row r as exact-bf16 quadratic pairs (h=r>>3, l=r&7) so the one-hot
    mask is folded into the edge matmul (rank-10) and exp(-t') yields the
    masked edge weight directly (mismatched rows get penalty >= 32 in the
    exponent -> exp ~ 0; padding slots get +30000).
  * Device phase A: pre_sup + per-node scalars via TensorE from
    host-transposed x @ W_aug (W_aug carries W | W@fw2 | W@fw1), batched
    4 windows per iteration; builds a 256B/row bf16 node table
    [vj(64) | l | b | 1 | b*l | 0...] and per-node row-side vector
    [a+fb | l | (a+fb)l | 1]; AllGather of the node table.
  * Phase B per (sweep, owner): SWDGE dma_gather (int16, owner-local)
    fetches per-edge source rows; prep/trigger/publish all run in
    Pool-only critical sections so descriptor-gen overlaps the previous
    gather's drain and the tile compute; per 128-edge tile ONE rank-10
    TensorE matmul produces the masked exponent grid, ACT exp gives the
    masked weights, and a second TensorE matmul (lhsT=masked weights,
    rhs=vj) accumulates the segment sum in a persistent PSUM block
    [128, 14 windows, 64]; ReLU + store at the end of the sweep.
"""

import sys

for _p in ("/opt/trn_rl_repo", "/opt/pypackages"):
    if _p not in sys.path:
        sys.path.append(_p)

import numpy as np
import ml_dtypes

import concourse.bass as bass
import concourse.bacc as bacc
import concourse.mybir as mybir
import concourse.tile as tile
from concourse.bass_utils import run_bass_kernel_spmd
from concourse.masks import make_identity

BF16 = ml_dtypes.bfloat16
P = 128
N_CORES = 8
ROWB = 128          # table row: 128 bf16 = 256B
CHUNK = 4           # tiles per exp batch
WGRP = 8            # windows per phase-A iteration
MPEN = 32.0         # mask mismatch penalty scale (exact in bf16)
MBIG = 30000.0      # padding-slot penalty


# ----------------------------------------------------------------- host prep

def host_prep(row, col, n_nodes, n_cores):
    npc = n_nodes // n_cores
    npad = ((npc + P - 1) // P) * P
    n_win = npad // P
    n_sweep = 7 if n_win % 7 == 0 else (4 if n_win % 4 == 0 else 2)
    assert n_win % n_sweep == 0
    hw = n_win // n_sweep                            # windows per sweep

    row = np.asarray(row).astype(np.int64)
    col = np.asarray(col).astype(np.int64)
    order = np.argsort(row, kind="stable")
    row_s = row[order].astype(np.int32)
    col_s = col[order].astype(np.int32)

    percore = []
    cnt = np.zeros((n_cores, n_cores, n_win), np.int64)   # [core, owner, window]
    for c in range(n_cores):
        base = c * npc
        lo = np.searchsorted(row_s, base, "left")
        hi = np.searchsorted(row_s, base + npc, "left")
        r = (row_s[lo:hi] - base).astype(np.int32)
        cc = col_s[lo:hi]
        o = (cc // npc).astype(np.int32)
        w = r // P
        np.add.at(cnt[c], (o, w), 1)
        percore.append((r, cc, o, w))
    # uniform tiles per (owner, window) bucket across cores
    B = np.maximum((cnt.max(axis=0) + P - 1) // P, 1)     # [owner, window]
    n_tiles = int(B.sum())
    n_slots = n_tiles * P
    # slot offset of bucket (o, w): layout [sweep][owner][window][tiles]
    tile_base = np.zeros((n_cores, n_win), np.int64)
    t0 = 0
    for hh in range(n_sweep):
        for oo in range(n_cores):
            for wi in range(hh * hw, (hh + 1) * hw):
                tile_base[oo, wi] = t0
                t0 += int(B[oo, wi])
    assert t0 == n_tiles

    shards = []
    for c in range(n_cores):
        r, cc, o, w = percore[c]
        idx16 = np.zeros(n_slots, np.int16)
        rloc = np.full(n_slots, -1, np.int32)
        bo = np.lexsort((cc, w, o))                  # sort by (owner, window, col)
        r, cc, o, w = r[bo], cc[bo], o[bo], w[bo]
        key = o.astype(np.int64) * n_win + w
        starts = np.searchsorted(key, np.arange(n_cores * n_win))
        ends = np.searchsorted(key, np.arange(n_cores * n_win), "right")
        for oo in range(n_cores):
            for wi in range(n_win):
                a, b = int(starts[oo * n_win + wi]), int(ends[oo * n_win + wi])
                if a == b:
                    continue
                s0 = int(tile_base[oo, wi]) * P
                k = b - a
                idx16[s0:s0 + k] = (cc[a:b] % npc).astype(np.int16)
                rloc[s0:s0 + k] = (r[a:b] - wi * P)
        deg = (np.bincount(r, minlength=npad) + 1).astype(np.float32)
        # mask features [6, n_slots]: [Mh^2, -2Mh, M, Ml^2, -2Ml, M];
        # padding slots get [MBIG, 0, 0, 0, 0, 0]
        pad = rloc < 0
        rr = np.where(pad, 0, rloc)
        h = (rr >> 3).astype(np.float32)
        l = (rr & 7).astype(np.float32)
        vm = np.stack([
            MPEN * h * h, -2.0 * MPEN * h,
            np.full(n_slots, MPEN, np.float32),
            MPEN * l * l, -2.0 * MPEN * l,
            np.full(n_slots, MPEN, np.float32),
        ], axis=0)
        vm[0, pad] = MBIG
        vm[1:, pad] = 0.0
        shards.append(dict(
            idx16=np.tile(np.ascontiguousarray(
                idx16.reshape(n_slots // 16, 16).T), (8, 1)),   # [128, S/16]
            vmask=np.ascontiguousarray(vm.astype(BF16)),        # [6, n_slots]
            deg=deg.reshape(npad, 1),
        ))
    L = dict(npc=npc, npad=npad, n_win=n_win, hw=hw, n_sweep=n_sweep, B=B,
             tile_base=tile_base, n_tiles=n_tiles)
    return shards, L


# ------------------------------------------------------------- device program

def build_program(L, in_dim, out_dim, n_cores):
    npad, n_win, hw = L["npad"], L["n_win"], L["hw"]
    n_sweep = L["n_sweep"]
    B, tile_base, n_tiles = L["B"], L["tile_base"], L["n_tiles"]
    n_k = in_dim // P
    f32, bf16, i16 = mybir.dt.float32, mybir.dt.bfloat16, mybir.dt.int16

    nc = bacc.Bacc("TRN2", target_bir_lowering=False, debug=False,
                   num_devices=n_cores)

    xint = nc.declare_dram_parameter("xint", [in_dim, npad], f32, isOutput=False)
    degp = nc.declare_dram_parameter("deg", [npad, 1], f32, isOutput=False)
    wmat = nc.declare_dram_parameter("wmat", [in_dim, out_dim], f32, isOutput=False)
    fw12 = nc.declare_dram_parameter("fw12", [out_dim, 2], f32, isOutput=False)
    fbrep = nc.declare_dram_parameter("fbrep", [P, 1], f32, isOutput=False)
    idxp = nc.declare_dram_parameter("idx16", [P, n_tiles * P // 16], i16,
                                     isOutput=False)
    vmaskp = nc.declare_dram_parameter("vmask", [6, n_tiles * P], bf16,
                                       isOutput=False)
    u6p = nc.declare_dram_parameter("u6rep", [6, hw * P], bf16, isOutput=False)
    outp = nc.declare_dram_parameter("out", [npad, out_dim], f32, isOutput=True)
    idx_cols = n_tiles * P // 16

    with tile.TileContext(nc) as tc:
        with (
            tc.tile_pool(name="dram", bufs=1, space="DRAM") as dpool,
            tc.tile_pool(name="const", bufs=1) as cpool,
        ):
            t_loc = dpool.tile([npad, ROWB], bf16)
            t2_loc = dpool.tile([npad, 4], bf16)
            t_glob = dpool.tile([n_cores * npad, ROWB], bf16)

            identity = cpool.tile([P, P], bf16)
            make_identity(nc, identity[:])
            fb_sb = cpool.tile([P, 1], f32)
            nc.sync.dma_start(out=fb_sb[:], in_=fbrep[:, :])
            fw_sb = cpool.tile([out_dim, 2], f32)
            nc.sync.dma_start(out=fw_sb[:], in_=fw12[:, :])
            fw_bf = cpool.tile([out_dim, 2], bf16)
            nc.vector.tensor_copy(fw_bf[:], fw_sb[:])

            # W_aug = [W | W@fw2 | W@fw1]  bf16 [P, n_k, 66]
            w_aug = cpool.tile([P, n_k, 66], bf16)
            with (
                tc.tile_pool(name="wtmp", bufs=2) as wpool,
                tc.tile_pool(name="wps", bufs=2, space="PSUM") as wps,
            ):
                wf = wpool.tile([P, n_k, out_dim], f32)
                nc.sync.dma_start(
                    out=wf[:], in_=wmat[:, :].rearrange("(s p) f -> p s f", p=P))
                nc.vector.tensor_copy(w_aug[:, :, 0:out_dim], wf[:])
                for s in range(n_k):
                    pT = wps.tile([out_dim, P], bf16, space="PSUM", tag="pT")
                    nc.tensor.transpose(out=pT[:], in_=w_aug[:, s, 0:out_dim],
                                        identity=identity[:])
                    wT = wpool.tile([out_dim, P], bf16, tag="wT")
                    nc.vector.tensor_copy(wT[:], pT[:])
                    pab = wps.tile([P, 2], f32, space="PSUM", tag="pab")
                    nc.tensor.matmul(out=pab[:], lhsT=wT[:], rhs=fw_bf[:],
                                     start=True, stop=True)
                    nc.vector.tensor_copy(w_aug[:, s, 64:65], pab[:, 1:2])  # b
                    nc.vector.tensor_copy(w_aug[:, s, 65:66], pab[:, 0:1])  # a

            # phase A (batched WGRP windows): T row [vj(64)|l|b|1|bl|0...];
            # T2 row [a'|l|a'l|1]
            groups = []
            w0 = 0
            while w0 < n_win:
                wn = min(WGRP, n_win - w0)
                groups.append((w0, wn))
                w0 += wn
            with (
                tc.tile_pool(name="xa", bufs=3) as xa,
                tc.tile_pool(name="psa", bufs=2, space="PSUM") as psa,
            ):
                for (g0, gn) in groups:
                    xf = xa.tile([P, n_k, WGRP * P], f32, tag="xf")
                    for s in range(n_k):
                        nc.sync.dma_start(
                            out=xf[:, s, 0:gn * P],
                            in_=xint[s * P:(s + 1) * P, g0 * P:(g0 + gn) * P])
                    xb = xa.tile([P, n_k, WGRP * P], bf16, tag="xb")
                    nc.vector.tensor_copy(xb[:, :, 0:gn * P], xf[:, :, 0:gn * P])
                    pt4 = psa.tile([P, WGRP, 128], f32, space="PSUM",
                                   tag="pt4")
                    pt = pt4[:, :, 0:66]
                    for w in range(gn):
                        for s in range(n_k):
                            nc.tensor.matmul(
                                out=pt4[:, w, 0:66],
                                lhsT=xb[:, s, w * P:(w + 1) * P],
                                rhs=w_aug[:, s, :],
                                start=(s == 0), stop=(s == n_k - 1))
                    dg = xa.tile([P, WGRP, 1], f32, tag="dg")
                    nc.sync.dma_start(
                        out=dg[:, 0:gn, :],
                        in_=degp[g0 * P:(g0 + gn) * P, :].rearrange(
                            "(w p) o -> p w o", p=P))
                    ldg = xa.tile([P, WGRP, 1], f32, tag="ldg")
                    nc.scalar.activation(ldg[:, 0:gn, :], dg[:, 0:gn, :],
                                         mybir.ActivationFunctionType.Ln)
                    bl = xa.tile([P, WGRP, 1], f32, tag="bl")
                    nc.vector.tensor_mul(bl[:, 0:gn, :], pt[:, 0:gn, 64:65],
                                         ldg[:, 0:gn, :])
                    tt = xa.tile([P, WGRP, ROWB], bf16, tag="tt")
                    nc.vector.memset(tt[:, 0:gn, 68:ROWB], 0.0)
                    nc.vector.tensor_copy(tt[:, 0:gn, 0:64], pt[:, 0:gn, 0:64])
                    nc.vector.tensor_copy(tt[:, 0:gn, 64:65], ldg[:, 0:gn, :])
                    nc.vector.tensor_copy(tt[:, 0:gn, 65:66], pt[:, 0:gn, 64:65])
                    nc.vector.memset(tt[:, 0:gn, 66:67], 1.0)
                    nc.vector.tensor_copy(tt[:, 0:gn, 67:68], bl[:, 0:gn, :])
                    ap_ = xa.tile([P, WGRP, 1], f32, tag="ap_")
                    nc.vector.tensor_tensor(
                        out=ap_[:, 0:gn, :], in0=pt[:, 0:gn, 65:66],
                        in1=fb_sb[:, None, :].to_broadcast([P, gn, 1]),
                        op=mybir.AluOpType.add)
                    al = xa.tile([P, WGRP, 1], f32, tag="al")
                    nc.vector.tensor_mul(al[:, 0:gn, :], ap_[:, 0:gn, :],
                                         ldg[:, 0:gn, :])
                    t2t = xa.tile([P, WGRP, 4], bf16, tag="t2t")
                    nc.vector.tensor_copy(t2t[:, 0:gn, 0:1], ap_[:, 0:gn, :])
                    nc.vector.tensor_copy(t2t[:, 0:gn, 1:2], ldg[:, 0:gn, :])
                    nc.vector.tensor_copy(t2t[:, 0:gn, 2:3], al[:, 0:gn, :])
                    nc.vector.memset(t2t[:, 0:gn, 3:4], 1.0)
                    nc.sync.dma_start(
                        out=t_loc[g0 * P:(g0 + gn) * P, :].rearrange(
                            "(w p) f -> p w f", p=P),
                        in_=tt[:, 0:gn, :])
                    nc.sync.dma_start(
                        out=t2_loc[g0 * P:(g0 + gn) * P, :].rearrange(
                            "(w p) f -> p w f", p=P),
                        in_=t2t[:, 0:gn, :])

            nc.gpsimd.collective_compute(
                "AllGather", mybir.AluOpType.bypass,
                replica_groups=[list(range(n_cores))],
                ins=[t_loc.opt()], outs=[t_glob.opt()],
            )

            # ---------------- phase B
            dma_sem = nc.alloc_semaphore("dg_dma")
            prep_sem = nc.alloc_semaphore("dg_prep")

            from contextlib import ExitStack
            with ExitStack() as _st:
                ub = _st.enter_context(tc.tile_pool(name="ub", bufs=2))
                ixp = _st.enter_context(tc.tile_pool(name="ixp", bufs=6))
                tg_pools = [
                    _st.enter_context(tc.tile_pool(name=f"tg{i}", bufs=1))
                    for i in range(9)]
                vbp = _st.enter_context(tc.tile_pool(name="vbp", bufs=6))
                wbp = _st.enter_context(tc.tile_pool(name="wb", bufs=3))
                mkp = _st.enter_context(tc.tile_pool(name="mkp", bufs=4))
                psb = _st.enter_context(
                    tc.tile_pool(name="psb", bufs=2, space="PSUM"))
                psc = _st.enter_context(
                    tc.tile_pool(name="psc", bufs=2, space="PSUM"))
                psd = _st.enter_context(
                    tc.tile_pool(name="psd", bufs=2, space="PSUM"))

                # flat cross-sweep block list: (sweep, owner, tid0, nt)
                all_blocks = []
                for hh in range(n_sweep):
                    for oo in range(n_cores):
                        t_lo = int(tile_base[oo, hh * hw])
                        nt_all = int(B[oo, hh * hw:(hh + 1) * hw].sum())
                        if nt_all:
                            all_blocks.append((hh, oo, t_lo, nt_all))
                NB = len(all_blocks)

                def wins_of(hh_, tid0_, nt_):
                    out = []
                    for oo_ in range(n_cores):
                        for wi_ in range(hw):
                            for _b in range(int(B[oo_, hh_ * hw + wi_])):
                                out.append(wi_)
                    base_ = int(tile_base[0, hh_ * hw])
                    return out[tid0_ - base_:tid0_ - base_ + nt_]

                # prep side: descriptor-gen + trigger in a Pool-only critical;
                # the publish (gpsimd wait for the gather DMA + dummy copies)
                # is its own tiny Pool critical issued BEFORE prep(j+2), so
                # compute(j) is released two criticals before any critical
                # whose entry barrier covers it: gen, drain and compute all
                # overlap.
                tg_tiles = [None] * NB

                GSZ = 3
                grp = [list(range(a, min(a + GSZ, NB)))
                       for a in range(0, NB, GSZ)]
                NG = len(grp)

                def _publish(js):
                    nc.gpsimd.wait_ge(dma_sem, 16 * (js[-1] + 1))
                    for j in js:
                        tgp = tg_tiles[j]
                        nc.gpsimd.tensor_copy(tgp[:, :, 63:64],
                                              tgp[:, :, 63:64])
                        nc.gpsimd.tensor_copy(tgp[:, :, 64:68],
                                              tgp[:, :, 64:68])

                def prep_group(gi):
                    ix_list = []
                    for j in grp[gi]:
                        hh, oo, tid0, nt = all_blocks[j]
                        n_idx = nt * P
                        tg_j = tg_pools[j % 9].tile(
                            [P, nt, ROWB], bf16, tag="tg")
                        tg_tiles[j] = tg_j
                        ixs = ixp.tile([P, n_idx // 16], i16, tag="ixs")
                        c0 = tid0 * P // 16
                        nc.sync.dma_start(
                            out=ixs[:], in_=idxp[:, c0:c0 + n_idx // 16])
                        ix_list.append(ixs)
                    with tc.tile_critical(no_gpsimd_drain=True):
                        for k, j in enumerate(grp[gi]):
                            hh, oo, tid0, nt = all_blocks[j]
                            n_idx = nt * P
                            nc.gpsimd.dma_gather(
                                out_ap=tg_tiles[j][:],
                                in_ap=t_glob[oo * npad:(oo + 1) * npad, :],
                                idxs_ap=ix_list[k][:],
                                num_idxs=n_idx, num_idxs_reg=n_idx,
                                elem_size=ROWB, single_packet=False,
                                prepare_only=True, sem=dma_sem,
                            ).then_inc(prep_sem, 1)
                            nc.gpsimd.wait_ge(prep_sem, j + 1)
                            nc.gpsimd.trigger_dma(count=1)
                        if gi >= 1:
                            _publish(grp[gi - 1])

                def publish_group(gi):
                    with tc.tile_critical(no_gpsimd_drain=True):
                        _publish(grp[gi])

                sweep_state = {}

                def sweep_setup(hh):
                    u_sb = ub.tile([10, hw * P], bf16, tag="u_sb")
                    nc.sync.dma_start(out=u_sb[4:10, :], in_=u6p[:, :])
                    for wi in range(hw):
                        gwin = hh * hw + wi
                        t2w = wbp.tile([P, 4], bf16, tag="t2w")
                        nc.sync.dma_start(
                            out=t2w[:], in_=t2_loc[gwin * P:(gwin + 1) * P, :])
                        put = psd.tile([CHUNK * 4, P], bf16, space="PSUM", tag="ptr")
                        nc.tensor.transpose(out=put[0:4, :], in_=t2w[:],
                                            identity=identity[:])
                        nc.scalar.copy(u_sb[0:4, wi * P:(wi + 1) * P], put[0:4, :])
                    po = psb.tile([P, hw, out_dim], f32, space="PSUM", tag="po")
                    nc.vector.memset(po[:], 0.0)
                    sweep_state[hh] = (u_sb, po)

                def sweep_store(hh):
                    _, po = sweep_state[hh]
                    for wi in range(hw):
                        gwin = hh * hw + wi
                        ob = wbp.tile([P, out_dim], f32, tag="ob")
                        nc.scalar.activation(ob[:], po[:, wi, :],
                                             mybir.ActivationFunctionType.Relu)
                        nc.scalar.dma_start(
                            out=outp[gwin * P:(gwin + 1) * P, :], in_=ob[:])

                def process_tiles(hh, tg, nt, tid0, wins, v10):
                    u_sb, po = sweep_state[hh]
                    for ch0 in range(0, nt, CHUNK):
                        m = min(CHUNK, nt - ch0)
                        pt_ = psc.tile([P, CHUNK, P], f32, space="PSUM",
                                       tag="pt_")
                        for q in range(m):
                            pvt = psd.tile([4, P], bf16,
                                           space="PSUM", tag="ptr")
                            nc.tensor.transpose(
                                out=pvt[:],
                                in_=tg[:, ch0 + q, 64:68],
                                identity=identity[:])
                            dst = v10[0:4, (ch0 + q) * P:(ch0 + q + 1) * P]
                            nc.scalar.copy(dst, pvt[:])
                        for q in range(m):
                            wq = wins[ch0 + q]
                            nc.tensor.matmul(
                                out=pt_[:, q, :],
                                lhsT=v10[:, (ch0 + q) * P:(ch0 + q + 1) * P],
                                rhs=u_sb[:, wq * P:(wq + 1) * P],
                                start=True, stop=True)
                        # exp(-t') IS the masked weight grid
                        msk = mkp.tile([P, CHUNK, P], bf16, tag="msk")
                        nc.scalar.activation(
                            msk[:, 0:m, :], pt_[:, 0:m, :],
                            mybir.ActivationFunctionType.Exp, scale=-1.0)
                        for q in range(m):
                            wq = wins[ch0 + q]
                            nc.tensor.matmul(
                                out=po[:, wq, :],
                                lhsT=msk[:, q, :],
                                rhs=tg[:, ch0 + q, 0:64],
                                start=False, stop=False,
                                skip_group_check=True)

                prep_group(0)
                v10_t = {}
                pending_stores = []
                for gi in range(NG):
                    for (fgi, hh_s) in [p for p in pending_stores
                                        if p[0] <= gi]:
                        sweep_store(hh_s)
                    pending_stores = [p for p in pending_stores if p[0] > gi]
                    for j in grp[gi]:
                        hh, oo, tid0, nt = all_blocks[j]
                        v10 = vbp.tile([10, nt * P], bf16, tag="v10")
                        nc.sync.dma_start(
                            out=v10[4:10, :],
                            in_=vmaskp[:, tid0 * P:(tid0 + nt) * P])
                        v10_t[j] = v10
                    if gi + 1 < NG:
                        prep_group(gi + 1)     # also publishes group gi
                    else:
                        publish_group(gi)
                    for j in grp[gi]:
                        hh, oo, tid0, nt = all_blocks[j]
                        if oo == 0:
                            sweep_setup(hh)
                        process_tiles(hh, tg_tiles[j], nt, tid0,
                                      wins_of(hh, tid0, nt), v10_t.pop(j))
                        tg_tiles[j] = None
                        if oo == n_cores - 1:
                            pending_stores.append((gi + 2, hh))
                for (fgi, hh_s) in pending_stores:
                    sweep_store(hh_s)

    nc.compile()
    return nc


# ------------------------------------------------------------------ assemble

def make_in_maps(x, W_, f_w, f_b, shards, L, n_cores):
    npc, npad, in_dim = L["npc"], L["npad"], x.shape[1]
    hw = L["hw"]
    fw12 = np.stack([f_w[:64, 0], f_w[64:, 0]], axis=1).astype(np.float32)
    fbrep = np.full((P, 1), np.float32(f_b[0]), np.float32)
    # u-side mask features per window row r: [1, h, h^2, 1, l, l^2]
    r = np.arange(P, dtype=np.float32)
    h = np.floor(r / 8.0)
    l = r - 8.0 * h
    u6 = np.stack([np.ones(P, np.float32), h, h * h,
                   np.ones(P, np.float32), l, l * l], axis=0)
    u6rep = np.ascontiguousarray(np.tile(u6, (1, hw)).astype(BF16))
    in_maps = []
    for c in range(n_cores):
        xsh = np.zeros((npad, in_dim), np.float32)
        xsh[:npc] = x[c * npc:(c + 1) * npc]
        xint = np.ascontiguousarray(xsh.T)
        in_maps.append({
            "xint": xint,
            "deg": shards[c]["deg"],
            "wmat": np.ascontiguousarray(W_, np.float32),
            "fw12": fw12,
            "fbrep": fbrep,
            "idx16": shards[c]["idx16"],
            "vmask": shards[c]["vmask"],
            "u6rep": u6rep,
        })
    return in_maps


def kernel(x, W, f_w, f_b, row, col, _profile=None):
    x = np.asarray(x, np.float32)
    W = np.asarray(W, np.float32)
    f_w = np.asarray(f_w, np.float32)
    f_b = np.asarray(f_b, np.float32)
    n = x.shape[0]

    shards, L = host_prep(row, col, n, N_CORES)
    nc = build_program(L, x.shape[1], 64, N_CORES)
    in_maps = make_in_maps(x, W, f_w, f_b, shards, L, N_CORES)
    res = run_bass_kernel_spmd(
        nc, in_maps, core_ids=list(range(N_CORES)), trace=_profile is not None)
    if _profile is not None and isinstance(_profile, dict):
        _profile["exec_time_ns"] = res.exec_time_ns
        _profile["mean_exec_time_ns"] = res.mean_exec_time_ns

    npc = L["npc"]
    out = np.empty((n, 64), np.float32)
    for c in range(N_CORES):
        out[c * npc:(c + 1) * npc] = res.results[c]["out"][:npc]
    return out
